# revision 1
# baseline (speedup 1.0000x reference)
"""Trainium2 Bass kernel for CustomGRU (B=64, T=512, D=512, U=1024).

Sharding: data-parallel over batch across 8 NeuronCores (8 rows each),
weights replicated (a per-step inter-core exchange is ruled out by the
~5-12us collective latency floor x 512 steps). Per core:

  Phase 1 (projections): xzr[t,b,:] = X[b,t,:] @ [Wz|Wr|Wh] + b  -> DRAM
    - stationary = X^T tiles (host-pre-transposed), moving = Wcat (f32r,
      1 cyc/row at N=512).
  Phase 2 (recurrence), per step t, all in B-major [8, u] except the
  matmul stationaries:
    - gate pre-activations h_{t-1} @ U via 4-way column-tiled PE
      streaming: h^T chunks [128,8] (zero-padded to M=32 slots) are
      stationary in four 32-column groups of the PE array
      (tile_position=(0,32g)); the fp16 U-weight slices [128,512] stream
      through 4 XBUSes concurrently, 2 rounds of 4 chunks accumulating
      into partition blocks 32g..32g+8 of one PSUM tile. Round-1 matmuls
      use start=True (the has_written clear is region-scoped). An
      "eye-matmul" accumulates xzr_t (kept f32r for precision) onto
      group 0. A copy + ones-pattern matmul reduces the 4 partition
      blocks to the [8,512] gate pre-activation.
    - sigmoid/tanh on ScalarE evict the reduced PSUM -> SBUF.
    - r is PE-transposed to U-major to form (r*h)^T, the stationary of
      the candidate matmul; h_new is PE-transposed back to h^T.
    - combine h = hh + z*(h_prev - hh) on VectorE; the tanh/combine/
      transpose/copy tail is split into 512-halves so the next step's
      round-1 matmuls (needing only h chunks 0-3) start early.

Weight matmuls run in fp16 (1 cyc/row, col-tiling compatible; ~2e-4
end-to-end rel err, same order as float32r); reductions, xz preloads
and projections in f32r; everything else fp32.
"""
import sys

if "/opt/trn_rl_repo" not in sys.path:
    sys.path.insert(0, "/opt/trn_rl_repo")

import numpy as np
from contextlib import ExitStack

import concourse.bass as bass
import concourse.bacc as bacc
import concourse.tile as tile
from concourse import mybir
from concourse.bass_utils import run_bass_kernel_spmd

F32 = mybir.dt.float32
F32R = mybir.dt.float32r
F16 = mybir.dt.float16

N_CORES = 8
B = 64
BS = B // N_CORES  # 8 batch rows per core
D = 512
U = 1024
U3 = 3 * U        # 3072 (z|r|h)
KC = U // 128     # 8 contraction chunks of 128
DC = D // 128     # 4 input-dim chunks


def build(nc, T, reps=1):
    BT = BS * T

    # ---- DRAM I/O (per-core) ----
    xT_d = nc.dram_tensor("xT", [D, BT], F32R, kind="ExternalInput")
    wcat_d = nc.dram_tensor("wcat", [D, U3], F32R, kind="ExternalInput")
    bb_d = nc.dram_tensor("bb", [128, U3], F32, kind="ExternalInput")
    uzr_d = nc.dram_tensor("uzr", [U, 2 * U], F16, kind="ExternalInput")
    uh_d = nc.dram_tensor("uh", [U, U], F16, kind="ExternalInput")
    eye8r_d = nc.dram_tensor("eye8r", [BS, BS], F32R, kind="ExternalInput")
    ones4_d = nc.dram_tensor("ones4", [128, BS], F32R, kind="ExternalInput")
    eye8f_d = nc.dram_tensor("eye8f", [BS, BS], F32, kind="ExternalInput")
    out_d = nc.dram_tensor("out", [T, BS, U], F32, kind="ExternalOutput")

    with tile.TileContext(nc) as tc, ExitStack() as ctx:
        dram = ctx.enter_context(tc.tile_pool(name="dram", bufs=1, space="DRAM"))
        xzr_d = dram.tile([T, BS, U3], F32R)

        const = ctx.enter_context(tc.tile_pool(name="const", bufs=1))
        eye8r = const.tile([BS, BS], F32R)
        nc.sync.dma_start(eye8r[:], eye8r_d[:])
        ones4 = const.tile([128, BS], F32R)
        nc.sync.dma_start(ones4[:], ones4_d[:])
        eye8f = const.tile([BS, BS], F32)
        nc.sync.dma_start(eye8f[:], eye8f_d[:])

        # ---------------- Phase 1: input projections ----------------
        with ExitStack() as p1:
            wpool = p1.enter_context(tc.tile_pool(name="wcat", bufs=1))
            wcat = wpool.tile([128, DC * U3], F32R)  # [p, dc, u]
            nc.sync.dma_start(
                wcat[:].rearrange("p (dc u) -> p dc u", dc=DC),
                wcat_d.rearrange("(dc p) u -> p dc u", p=128),
            )
            bb = wpool.tile([128, U3], F32)
            nc.sync.dma_start(bb[:], bb_d[:])

            xp = p1.enter_context(tc.tile_pool(name="xT", bufs=3))
            op = p1.enter_context(tc.tile_pool(name="p1out", bufs=3))
            pp = p1.enter_context(tc.tile_pool(name="p1ps", bufs=4, space="PSUM"))

            n_bt = BT // 128          # bt-chunks of 128 (4 per batch row)
            tpb = T // 128            # t-chunks per batch row
            for tb in range(n_bt):
                b_idx, t_blk = tb // tpb, tb % tpb
                xt = xp.tile([128, DC * 128], F32R, tag="xt")  # [p=d, dc, bt]
                nc.sync.dma_start(
                    xt[:].rearrange("p (dc n) -> p dc n", dc=DC),
                    xT_d[:, tb * 128:(tb + 1) * 128].rearrange(
                        "(dc p) n -> p dc n", p=128
                    ),
                )
                for ut in range(U3 // 512):
                    ps = pp.tile([128, 512], F32, tag="ps")
                    for dc in range(DC):
                        nc.tensor.matmul(
                            ps[:],
                            xt[:, dc * 128:(dc + 1) * 128],
                            wcat[:, dc * U3 + ut * 512: dc * U3 + ut * 512 + 512],
                            start=(dc == 0),
                            stop=(dc == DC - 1),
                        )
                    ob = op.tile([128, 512], F32R, tag="ob")
                    nc.vector.tensor_add(
                        ob[:], ps[:], bb[:, ut * 512:(ut + 1) * 512]
                    )
                    nc.sync.dma_start(
                        xzr_d[
                            t_blk * 128:(t_blk + 1) * 128,
                            b_idx,
                            ut * 512:(ut + 1) * 512,
                        ].squeeze(),
                        ob[:],
                    )

        # ---------------- Phase 2: recurrence ----------------
        upool = ctx.enter_context(tc.tile_pool(name="u", bufs=1))
        uzr = upool.tile([128, KC * 2 * U], F16)  # [p, k, 2U]
        nc.sync.dma_start(
            uzr[:].rearrange("p (k u) -> p k u", k=KC),
            uzr_d.rearrange("(k p) u -> p k u", p=128),
        )
        uh = upool.tile([128, KC * U], F16)
        nc.sync.dma_start(
            uh[:].rearrange("p (k u) -> p k u", k=KC),
            uh_d.rearrange("(k p) u -> p k u", p=128),
        )

        hpool = ctx.enter_context(tc.tile_pool(name="h", bufs=2))
        stage = ctx.enter_context(tc.tile_pool(name="stage", bufs=4))
        gates = ctx.enter_context(tc.tile_pool(name="gates", bufs=2))
        psg = ctx.enter_context(tc.tile_pool(name="psg", bufs=4, space="PSUM"))
        ps2 = ctx.enter_context(tc.tile_pool(name="ps2", bufs=2, space="PSUM"))
        pst = ctx.enter_context(tc.tile_pool(name="pst", bufs=1, space="PSUM"))
        red = ctx.enter_context(tc.tile_pool(name="red", bufs=3))
        # zero the col-tiled psum slots once so untouched partition rows
        # (multiplied by 0 in the ones-reduction) are never uninitialized
        for _i in range(4):
            _d = psg.tile([128, 512], F32, tag="psg")
            nc.vector.memset(_d[:], 0.0)

        # h^T chunks in 32-col padded slots (cols 32k..32k+8 hold chunk k,
        # rest zero) so col-tiled matmuls use M=32 stationaries.
        hT0 = const.tile([128, KC * 32], F16)
        nc.any.memzero(hT0[:])
        hT_prev = hT0
        # pre-zero the rT/hTps psum slots once: per-step transposes write
        # only the 8 valid cols of each 32-col slot; the full-width mul/copy
        # reads the (zero) pads
        _c = pst.tile([128, KC * 32], F32, tag="rT")
        nc.vector.memset(_c[:], 0.0)
        _e = pst.tile([128, KC * 32], F32, tag="hTps")
        nc.vector.memset(_e[:], 0.0)

        def gate_mms(xoff, uoff, umat, hT, tag, copy_eng):
            """Two [8,512] reduced psum tiles via 4-way col-tiled streaming.

            Each [8,512] gate tile: eye-MM preloads xz into partitions 0-8
            (start=True clears the bank), then 8 K-chunk matmuls run on 4
            col-groups (tile_position=(0,32g), 2 rounds) writing partials to
            partition blocks 32g..32g+8. A copy + ones-pattern matmul sums
            the 4 blocks (+xz) back to [8,512]."""
            tiles = []
            for j in range(2):
                ps = psg.tile([128, 512], F32, tag="psg")
                xz = xz_t[:, xoff + 512 * j: xoff + 512 * j + 512]
                # round 1 (chunks 0-3): start=True so each group clears its
                # own 32-row psum region (has_written clear is region-scoped)
                for k in range(KC):
                    g = k % 4
                    nc.tensor.matmul(
                        ps[32 * g:32 * g + 32, :],
                        hT[:, k * 32:(k + 1) * 32],
                        umat[:, k * WSTRIDE + uoff + 512 * j:
                             k * WSTRIDE + uoff + 512 * j + 512],
                        start=(k < 4),
                        stop=(k == KC - 1),
                        tile_position=(0, 32 * g),
                        skip_group_check=True,
                    )
                    if k == 3:
                        # xz preload accumulates onto group 0's rows 0-8
                        nc.tensor.matmul(ps[0:BS, :], eye8r[:], xz,
                                         start=False, stop=False,
                                         tile_position=(0, 0),
                                         skip_group_check=True)
                sb = red.tile([128, 512], F32R, tag="red")
                if copy_eng == "act":
                    nc.scalar.copy(sb[:], ps[:])
                else:
                    nc.vector.tensor_copy(sb[:], ps[:])
                pr = ps2.tile([BS, 512], F32, tag="ps2")
                nc.tensor.matmul(pr[:], ones4[:], sb[:], start=True, stop=True)
                tiles.append(pr)
            return tiles

        for rep in range(reps):
          for t in range(T):
            xz_t = stage.tile([BS, U3], F32R, tag="xz")
            nc.sync.dma_start(xz_t[:], xzr_d[t].squeeze())

            # r gate, then transpose to U-major and form (r*h)^T
            WSTRIDE = 2 * U
            ps_r = gate_mms(U, U, uzr, hT_prev, "r", "dve")
            # z gate (keeps PE busy while sigmoid(r) runs)
            ps_z = gate_mms(0, 0, uzr, hT_prev, "z", "act")
            r_B = gates.tile([BS, U], F32, tag="r")
            rT = pst.tile([128, KC * 32], F32, tag="rT")
            rhT = hpool.tile([128, KC * 32], F16, tag="rhT")
            for j in range(2):
                sl = slice(512 * j, 512 * j + 512)
                nc.scalar.activation(
                    r_B[:, sl], ps_r[j][:],
                    mybir.ActivationFunctionType.Sigmoid,
                )
                for c in range(4 * j, 4 * j + 4):
                    nc.tensor.transpose(
                        rT[:, c * 32:c * 32 + BS],
                        r_B[:, c * 128:(c + 1) * 128],
                        eye8f[:],
                    )
                nc.vector.tensor_mul(
                    rhT[:, 128 * j:128 * (j + 1)],
                    rT[:, 128 * j:128 * (j + 1)],
                    hT_prev[:, 128 * j:128 * (j + 1)])
            z_B = gates.tile([BS, U], F32, tag="z")
            for j in range(2):
                nc.scalar.activation(
                    z_B[:, 512 * j:512 * j + 512], ps_z[j][:],
                    mybir.ActivationFunctionType.Sigmoid,
                )

            # candidate
            WSTRIDE = U
            ps_h = gate_mms(2 * U, 0, uh, rhT, "hh", "act")
            if t == 0:
                h_B_prev = gates.tile([BS, U], F32, tag="hB")
                nc.any.memzero(h_B_prev[:])
            hh_B = gates.tile([BS, U], F32, tag="hh")
            h_B = gates.tile([BS, U], F32, tag="hB")
            hT_ps = pst.tile([128, KC * 32], F32, tag="hTps")
            hT_new = hpool.tile([128, KC * 32], F16, tag="hT")
            # per 512-half: tanh -> combine -> transpose -> h^T copy, so the
            # next step's round-1 matmuls (which read only h chunks 0-3)
            # start while this half-1 is still combining
            for j in range(2):
                sl = slice(512 * j, 512 * j + 512)
                nc.scalar.activation(
                    hh_B[:, sl], ps_h[j][:],
                    mybir.ActivationFunctionType.Tanh,
                )
                # combine: h = hh + z * (h_prev - hh)   (B-major, VectorE)
                tmp = gates.tile([BS, 512], F32, tag="tmp")
                nc.vector.tensor_sub(tmp[:], h_B_prev[:, sl], hh_B[:, sl])
                nc.vector.tensor_mul(tmp[:], z_B[:, sl], tmp[:])
                nc.vector.tensor_add(h_B[:, sl], hh_B[:, sl], tmp[:])
                for c in range(4 * j, 4 * j + 4):
                    nc.tensor.transpose(
                        hT_ps[:, c * 32:c * 32 + BS],
                        h_B[:, c * 128:(c + 1) * 128],
                        eye8f[:],
                    )
                nc.vector.tensor_copy(
                    hT_new[:, 128 * j:128 * (j + 1)],
                    hT_ps[:, 128 * j:128 * (j + 1)])

            nc.sync.dma_start(out_d[t].squeeze(), h_B[:])
            hT_prev = hT_new
            h_B_prev = h_B

    nc.compile()
    return nc


def prepare(inputs, Wz, Uz, bz, Wr, Ur, br, Wh, Uh, bh, T):
    """Build the Bass program and the per-core input maps."""
    x = np.asarray(inputs, dtype=np.float32)[:, :T, :]

    wcat = np.concatenate([Wz, Wr, Wh], axis=1).astype(np.float32)
    bcat = np.concatenate([bz, br, bh]).astype(np.float32)
    bb = np.ascontiguousarray(np.broadcast_to(bcat, (128, U3)))
    uzr = np.concatenate([Uz, Ur], axis=1).astype(np.float16)
    uh = np.asarray(Uh).astype(np.float16)
    eye8 = np.eye(BS, dtype=np.float32)
    ones4 = np.zeros((128, BS), dtype=np.float32)
    for g in range(4):
        for b in range(BS):
            ones4[32 * g + b, b] = 1.0

    nc = bacc.Bacc("TRN2", target_bir_lowering=False, debug=False,
                   num_devices=N_CORES)
    build(nc, T)

    in_maps = []
    for c in range(N_CORES):
        xc = x[c * BS:(c + 1) * BS]               # [BS, T, D]
        xT = np.ascontiguousarray(xc.reshape(BS * T, D).T)  # [D, BS*T]
        in_maps.append({
            "xT": xT, "wcat": wcat, "bb": bb, "uzr": uzr, "uh": uh,
            "eye8r": eye8, "eye8f": eye8, "ones4": ones4,
        })
    return nc, in_maps


def assemble(results):
    outs = []
    for c in range(N_CORES):
        o = results[c]["out"]                     # [T, BS, U]
        outs.append(np.ascontiguousarray(o.transpose(1, 0, 2)))
    return np.concatenate(outs, axis=0)           # [B, T, U]


def kernel(inputs, Wz, Uz, bz, Wr, Ur, br, Wh, Uh, bh, _T=None):
    T = inputs.shape[1] if _T is None else _T
    nc, in_maps = prepare(inputs, Wz, Uz, bz, Wr, Ur, br, Wh, Uh, bh, T)
    res = run_bass_kernel_spmd(nc, in_maps, list(range(N_CORES)))
    return assemble(res.results)



# revision 3
# speedup vs baseline: 6.0134x; 6.0134x over previous
"""Trainium2 Bass kernel for CustomGRU (B=64, T=512, D=512, U=1024).

Sharding: data-parallel over batch across 8 NeuronCores (8 rows each),
weights replicated. Everything runs in the TRANSPOSED (U-major) layout:
the hidden state lives as hT[u, b] tiles [128, (uc, b)] = [128, 64], so
every gate matmul has the WEIGHT chunk [128, 128] as the stationary and
an 8-wide hT / xT chunk as the moving tensor (out [128, 8] PSUM column
slices). The input projection W^T x_t is folded into the recurrence as
4 extra contraction chunks per gate (no phase-1 GEMM, no xz staging in
DRAM, no per-step input DMA), and the bias is preloaded with a single
[K=8] matmul per gate (lhsT = bias reshaped [8, 128], moving =
eye8 kron ones8).

Per step t (per core):
  PE : r-bias/x/h (97 mm) -> c-bias/x (33) -> z-bias/x/h (97) -> c-rh (64)
  Act: rs=sigmoid(ps_r) [f16], zs=sigmoid(ps_z) [f32], hh=tanh(ps_c)
  DVE: rh=rs*hT, w=1-zs, zh=zs*h_prev, m1=w*hh, hT'=m1+zh [f16],
       h=m1+zh [f32 -> 16-step history ring, DMA-flushed to DRAM]

All matmul moving operands are f16 (1 cyc/row); weights/x are f16,
combine math f32. Host assembles [128, T, 8, 8] -> [B, T, U].
"""
import sys

if "/opt/trn_rl_repo" not in sys.path:
    sys.path.insert(0, "/opt/trn_rl_repo")

import numpy as np
from contextlib import ExitStack

import concourse.bass as bass
import concourse.bacc as bacc
import concourse.tile as tile
from concourse import mybir
from concourse.bass_utils import run_bass_kernel_spmd

F32 = mybir.dt.float32
F16 = mybir.dt.float16
MULT = mybir.AluOpType.mult
ADD = mybir.AluOpType.add
SIG = mybir.ActivationFunctionType.Sigmoid
TANH = mybir.ActivationFunctionType.Tanh

N_CORES = 8
B = 64
BS = B // N_CORES  # 8 batch rows per core
D = 512
U = 1024
KC = U // 128      # 8 contraction chunks over U
DC = D // 128      # 4 contraction chunks over D
NUC = U // 128     # 8 output u-chunks
FLUSH = 16         # steps per output-DMA flush


def build(nc, T, reps=1):
    # ---- DRAM I/O (per-core; weights replicated, x sharded) ----
    uz_d = nc.dram_tensor("uz", [128, KC * U], F16, kind="ExternalInput")
    ur_d = nc.dram_tensor("ur", [128, KC * U], F16, kind="ExternalInput")
    uh_d = nc.dram_tensor("uh", [128, KC * U], F16, kind="ExternalInput")
    wz_d = nc.dram_tensor("wz", [128, DC * U], F16, kind="ExternalInput")
    wr_d = nc.dram_tensor("wr", [128, DC * U], F16, kind="ExternalInput")
    wh_d = nc.dram_tensor("wh", [128, DC * U], F16, kind="ExternalInput")
    xt_d = nc.dram_tensor("xt", [128, DC * T * BS], F16, kind="ExternalInput")
    b8_d = nc.dram_tensor("b8", [8, 3 * 128], F16, kind="ExternalInput")
    e8_d = nc.dram_tensor("e8", [8, NUC * BS], F16, kind="ExternalInput")
    out_d = nc.dram_tensor("out", [128, T * 64], F32, kind="ExternalOutput")

    with tile.TileContext(nc) as tc, ExitStack() as ctx:
        const = ctx.enter_context(tc.tile_pool(name="const", bufs=1))
        uz = const.tile([128, KC * U], F16)
        nc.sync.dma_start(uz[:], uz_d[:])
        ur = const.tile([128, KC * U], F16)
        nc.sync.dma_start(ur[:], ur_d[:])
        uh = const.tile([128, KC * U], F16)
        nc.sync.dma_start(uh[:], uh_d[:])
        wz = const.tile([128, DC * U], F16)
        nc.sync.dma_start(wz[:], wz_d[:])
        wr = const.tile([128, DC * U], F16)
        nc.sync.dma_start(wr[:], wr_d[:])
        wh = const.tile([128, DC * U], F16)
        nc.sync.dma_start(wh[:], wh_d[:])
        xt = const.tile([128, DC * T * BS], F16)
        nc.sync.dma_start(xt[:], xt_d[:])
        b8 = const.tile([8, 3 * 128], F16)
        nc.sync.dma_start(b8[:], b8_d[:])
        e8 = const.tile([8, NUC * BS], F16)
        nc.sync.dma_start(e8[:], e8_d[:])
        hT0 = const.tile([128, 64], F16)
        nc.vector.memset(hT0[:], 0.0)
        h00 = const.tile([128, 64], F32)
        nc.vector.memset(h00[:], 0.0)

        hpool = ctx.enter_context(tc.tile_pool(name="h", bufs=2))
        sp = ctx.enter_context(tc.tile_pool(name="s", bufs=2))
        histp = ctx.enter_context(tc.tile_pool(name="hist", bufs=2))
        psum = ctx.enter_context(tc.tile_pool(name="ps", bufs=2, space="PSUM"))

        def xgate(ps, wt, g):
            # bias preload (start=True clears the whole [128,64] tile), then
            # W^T x_t: 4 d-chunks x 8 u-chunks of out [128, 8]
            nc.tensor.matmul(ps[:], b8[:, g * 128:(g + 1) * 128], e8[:],
                             start=True, stop=False, skip_group_check=True)
            for dc in range(DC):
                xs = xt[:, dc * (T * BS) + t * BS: dc * (T * BS) + (t + 1) * BS]
                for uc in range(NUC):
                    nc.tensor.matmul(
                        ps[:, uc * 8:(uc + 1) * 8],
                        wt[:, dc * U + uc * 128: dc * U + (uc + 1) * 128],
                        xs, start=False, stop=False, skip_group_check=True)

        def hgate(ps, ut, mov):
            # U^T h: 8 k-chunks x 8 u-chunks; stop on the last write of
            # each column slice
            for k in range(KC):
                ms = mov[:, k * 8:(k + 1) * 8]
                for uc in range(NUC):
                    nc.tensor.matmul(
                        ps[:, uc * 8:(uc + 1) * 8],
                        ut[:, k * U + uc * 128: k * U + (uc + 1) * 128],
                        ms, start=False, stop=(k == KC - 1),
                        skip_group_check=True)

        hT_prev = hT0
        hprev = h00[:]
        hist = None
        for rep in range(reps):
          for t in range(T):
            if t % FLUSH == 0:
                hist = histp.tile([128, FLUSH * 64], F32, tag="hist")
            ps_r = psum.tile([128, 64], F32, tag="r")
            ps_z = psum.tile([128, 64], F32, tag="z")
            ps_c = psum.tile([128, 64], F32, tag="c")

            # PE stream: r first (critical path), then c bias+x, z, c-rh
            xgate(ps_r, wr, 1)
            hgate(ps_r, ur, hT_prev[:])
            xgate(ps_c, wh, 2)

            rs = sp.tile([128, 64], F16, tag="rs")
            nc.scalar.activation(rs[:], ps_r[:], SIG)
            rh = sp.tile([128, 64], F16, tag="rh")
            nc.vector.tensor_mul(rh[:], rs[:], hT_prev[:])

            xgate(ps_z, wz, 0)
            hgate(ps_z, uz, hT_prev[:])
            hgate(ps_c, uh, rh[:])

            zs = sp.tile([128, 64], F32, tag="zs")
            nc.scalar.activation(zs[:], ps_z[:], SIG)
            w = sp.tile([128, 64], F32, tag="w")
            nc.vector.tensor_scalar(w[:], zs[:], -1.0, 1.0, MULT, ADD)
            zh = sp.tile([128, 64], F32, tag="zh")
            nc.vector.tensor_mul(zh[:], zs[:], hprev)

            hh = sp.tile([128, 64], F32, tag="hh")
            nc.scalar.activation(hh[:], ps_c[:], TANH)
            m1 = sp.tile([128, 64], F32, tag="m1")
            nc.vector.tensor_mul(m1[:], w[:], hh[:])
            hT_new = hpool.tile([128, 64], F16, tag="hT")
            nc.vector.tensor_add(hT_new[:], m1[:], zh[:])
            hs = hist[:, (t % FLUSH) * 64: (t % FLUSH + 1) * 64]
            nc.vector.tensor_add(hs, m1[:], zh[:])
            if t % FLUSH == FLUSH - 1 or t == T - 1:
                base = t - (t % FLUSH)
                nc.sync.dma_start(
                    out_d[:, base * 64: (t + 1) * 64],
                    hist[:, : (t % FLUSH + 1) * 64])

            hT_prev = hT_new
            hprev = hs

    nc.compile()
    return nc


def _u_layout(M):
    # [U, U] -> [128, KC*U]: out[p, k*U+u] = M[k*128+p, u]
    return np.ascontiguousarray(
        np.asarray(M, np.float32).reshape(KC, 128, U).transpose(1, 0, 2)
        .reshape(128, KC * U)).astype(np.float16)


def _w_layout(M):
    # [D, U] -> [128, DC*U]: out[p, dc*U+u] = M[dc*128+p, u]
    return np.ascontiguousarray(
        np.asarray(M, np.float32).reshape(DC, 128, U).transpose(1, 0, 2)
        .reshape(128, DC * U)).astype(np.float16)


def prepare(inputs, Wz, Uz, bz, Wr, Ur, br, Wh, Uh, bh, T):
    """Build the Bass program and the per-core input maps."""
    x = np.asarray(inputs, dtype=np.float32)[:, :T, :]

    uz, ur, uh = _u_layout(Uz), _u_layout(Ur), _u_layout(Uh)
    wz, wr, wh = _w_layout(Wz), _w_layout(Wr), _w_layout(Wh)
    b8 = np.concatenate(
        [np.asarray(v, np.float32).reshape(8, 128) for v in (bz, br, bh)],
        axis=1).astype(np.float16)
    e8 = np.kron(np.eye(8, dtype=np.float16), np.ones((1, 8), np.float16))
    e8 = np.ascontiguousarray(e8)

    nc = bacc.Bacc("TRN2", target_bir_lowering=False, debug=False,
                   num_devices=N_CORES)
    build(nc, T)

    in_maps = []
    for c in range(N_CORES):
        xc = x[c * BS:(c + 1) * BS]               # [BS, T, D]
        # xt[p, dc, t, b] = xc[b, t, dc*128+p]
        xtc = np.ascontiguousarray(
            xc.reshape(BS, T, DC, 128).transpose(3, 2, 1, 0)
            .reshape(128, DC * T * BS)).astype(np.float16)
        in_maps.append({
            "uz": uz, "ur": ur, "uh": uh, "wz": wz, "wr": wr, "wh": wh,
            "xt": xtc, "b8": b8, "e8": e8,
        })
    return nc, in_maps


def assemble(results):
    outs = []
    T = results[0]["out"].shape[1] // 64
    for c in range(N_CORES):
        o = results[c]["out"]                     # [128, T*64]
        # o[p, t*64 + uc*8 + b] = h_t[b, uc*128+p]
        o = o.reshape(128, T, NUC, BS).transpose(3, 1, 2, 0)  # [b, t, uc, p]
        outs.append(np.ascontiguousarray(o.reshape(BS, T, U)))
    return np.concatenate(outs, axis=0)           # [B, T, U]


def kernel(inputs, Wz, Uz, bz, Wr, Ur, br, Wh, Uh, bh, _T=None):
    T = inputs.shape[1] if _T is None else _T
    nc, in_maps = prepare(inputs, Wz, Uz, bz, Wr, Ur, br, Wh, Uh, bh, T)
    res = run_bass_kernel_spmd(nc, in_maps, list(range(N_CORES)))
    return assemble(res.results)


# revision 5
# speedup vs baseline: 45.1545x; 7.5090x over previous
"""Trainium2 Bass kernel for CustomGRU (B=64, T=512, D=512, U=1024).

Sharding: data-parallel over batch across 8 NeuronCores (8 rows each),
weights replicated. Everything runs in the TRANSPOSED (U-major) layout:
the hidden state lives as hT[u, b] tiles [128, (uc, b)] = [128, 64], so
every gate matmul has the WEIGHT chunk [128, 128] as the stationary and
an 8-wide hT / xT chunk as the moving tensor (out [128, 8] PSUM column
slices). The input projection W^T x_t is folded into the recurrence as
4 extra contraction chunks per gate (no phase-1 GEMM, no xz staging in
DRAM, no per-step input DMA), and the bias is preloaded with a single
[K=8] matmul per gate (lhsT = bias reshaped [8, 128], moving =
eye8 kron ones8).

Per step t (per core):
  PE : r-bias/x/h (97 mm) -> c-bias/x (33) -> z-bias/x/h (97) -> c-rh (64)
  Act: rs=sigmoid(ps_r) [f16], zs=sigmoid(ps_z) [f32], hh=tanh(ps_c)
  DVE: rh=rs*hT, w=1-zs, zh=zs*h_prev, m1=w*hh, hT'=m1+zh [f16],
       h=m1+zh [f32 -> 16-step history ring, DMA-flushed to DRAM]

All matmul moving operands are f16 (1 cyc/row); weights/x are f16,
combine math f32. Host assembles [128, T, 8, 8] -> [B, T, U].
"""
import sys

if "/opt/trn_rl_repo" not in sys.path:
    sys.path.insert(0, "/opt/trn_rl_repo")

import numpy as np
from contextlib import ExitStack

import concourse.bass as bass
import concourse.bacc as bacc
import concourse.tile as tile
from concourse import mybir
from concourse.bass_utils import run_bass_kernel_spmd

F32 = mybir.dt.float32
F16 = mybir.dt.float16
MULT = mybir.AluOpType.mult
ADD = mybir.AluOpType.add
SIG = mybir.ActivationFunctionType.Sigmoid
TANH = mybir.ActivationFunctionType.Tanh

N_CORES = 8
B = 64
BS = B // N_CORES  # 8 batch rows per core
D = 512
U = 1024
KC = U // 128      # 8 contraction chunks over U
DC = D // 128      # 4 contraction chunks over D
NUC = U // 128     # 8 output u-chunks
FLUSH = 32         # steps per output-DMA flush


def build(nc, T, reps=1):
    # ---- DRAM I/O (per-core; weights replicated, x sharded) ----
    uz_d = nc.dram_tensor("uz", [128, KC * U], F16, kind="ExternalInput")
    ur_d = nc.dram_tensor("ur", [128, KC * U], F16, kind="ExternalInput")
    uh_d = nc.dram_tensor("uh", [128, KC * U], F16, kind="ExternalInput")
    wz_d = nc.dram_tensor("wz", [128, DC * U], F16, kind="ExternalInput")
    wr_d = nc.dram_tensor("wr", [128, DC * U], F16, kind="ExternalInput")
    wh_d = nc.dram_tensor("wh", [128, DC * U], F16, kind="ExternalInput")
    xt_d = nc.dram_tensor("xt", [128, DC * T * BS], F16, kind="ExternalInput")
    b8_d = nc.dram_tensor("b8", [8, 3 * 128], F16, kind="ExternalInput")
    e8_d = nc.dram_tensor("e8", [8, NUC * BS], F16, kind="ExternalInput")
    out_d = nc.dram_tensor("out", [128, T * 64], F32, kind="ExternalOutput")

    with tile.TileContext(nc) as tc, ExitStack() as ctx:
        const = ctx.enter_context(tc.tile_pool(name="const", bufs=1))
        uz = const.tile([128, KC * U], F16)
        nc.sync.dma_start(uz[:], uz_d[:])
        ur = const.tile([128, KC * U], F16)
        nc.sync.dma_start(ur[:], ur_d[:])
        uh = const.tile([128, KC * U], F16)
        nc.sync.dma_start(uh[:], uh_d[:])
        wz = const.tile([128, DC * U], F16)
        nc.sync.dma_start(wz[:], wz_d[:])
        wr = const.tile([128, DC * U], F16)
        nc.sync.dma_start(wr[:], wr_d[:])
        wh = const.tile([128, DC * U], F16)
        nc.sync.dma_start(wh[:], wh_d[:])
        xt = const.tile([128, DC * T * BS], F16)
        nc.sync.dma_start(xt[:], xt_d[:])
        b8 = const.tile([8, 3 * 128], F16)
        nc.sync.dma_start(b8[:], b8_d[:])
        e8 = const.tile([8, NUC * BS], F16)
        nc.sync.dma_start(e8[:], e8_d[:])
        hT0 = const.tile([128, 64], F16)
        nc.vector.memset(hT0[:], 0.0)
        h00 = const.tile([128, 64], F32)
        nc.vector.memset(h00[:], 0.0)

        hpool = ctx.enter_context(tc.tile_pool(name="h", bufs=2))
        sp = ctx.enter_context(tc.tile_pool(name="s", bufs=2))
        histp = ctx.enter_context(tc.tile_pool(name="hist", bufs=2))
        psum = ctx.enter_context(tc.tile_pool(name="ps", bufs=2, space="PSUM"))

        def xgate(ps, wt, g):
            # bias preload (start=True clears the whole [128,64] tile), then
            # W^T x_t: 4 d-chunks x 8 u-chunks of out [128, 8]
            nc.tensor.matmul(ps[:], b8[:, g * 128:(g + 1) * 128], e8[:],
                             start=True, stop=False, skip_group_check=True)
            for dc in range(DC):
                xs = xt[:, dc * (T * BS) + t * BS: dc * (T * BS) + (t + 1) * BS]
                for uc in range(NUC):
                    nc.tensor.matmul(
                        ps[:, uc * 8:(uc + 1) * 8],
                        wt[:, dc * U + uc * 128: dc * U + (uc + 1) * 128],
                        xs, start=False, stop=False, skip_group_check=True)

        def hgate(ps, ut, mov, stop=True):
            # U^T h: 8 k-chunks x 8 u-chunks; stop on the last write of
            # each column slice
            for k in range(KC):
                ms = mov[:, k * 8:(k + 1) * 8]
                for uc in range(NUC):
                    nc.tensor.matmul(
                        ps[:, uc * 8:(uc + 1) * 8],
                        ut[:, k * U + uc * 128: k * U + (uc + 1) * 128],
                        ms, start=False, stop=(stop and k == KC - 1),
                        skip_group_check=True)

        # r-gate consumes h = m1 + zh as two separate moving tensors, so
        # the critical path runs tanh -> m1 -> r-matmuls without waiting
        # for the materialized hT (which is produced off-path for z/rh).
        m1_prev = hT0
        zh_prev = hT0
        hT_prev = hT0
        hprev = h00[:]
        hist = None
        for rep in range(reps):
          for t in range(T):
            if t % FLUSH == 0:
                hist = histp.tile([128, FLUSH * 64], F32, tag="hist")
            ps_r = psum.tile([128, 64], F32, tag="r")
            ps_z = psum.tile([128, 64], F32, tag="z")
            ps_c = psum.tile([128, 64], F32, tag="c")

            # PE stream: r first (critical path: zh-part pre-runs, m1-part
            # is the only h-dependent leg), then c/z fill the gaps
            xgate(ps_r, wr, 1)
            hgate(ps_r, ur, zh_prev[:], stop=False)
            hgate(ps_r, ur, m1_prev[:])
            xgate(ps_c, wh, 2)

            rs = sp.tile([128, 64], F16, tag="rs")
            nc.scalar.activation(rs[:], ps_r[:], SIG)
            rh = sp.tile([128, 64], F16, tag="rh")
            nc.vector.tensor_mul(rh[:], rs[:], hT_prev[:])

            xgate(ps_z, wz, 0)
            hgate(ps_z, uz, hT_prev[:])
            hgate(ps_c, uh, rh[:])

            zs = sp.tile([128, 64], F32, tag="zs")
            nc.scalar.activation(zs[:], ps_z[:], SIG)
            w = sp.tile([128, 64], F32, tag="w")
            nc.vector.tensor_scalar(w[:], zs[:], -1.0, 1.0, MULT, ADD)
            zh = sp.tile([128, 64], F16, tag="zh")
            nc.vector.tensor_mul(zh[:], zs[:], hprev)

            hh = sp.tile([128, 64], F32, tag="hh")
            nc.scalar.activation(hh[:], ps_c[:], TANH)
            m1 = sp.tile([128, 64], F16, tag="m1")
            nc.vector.tensor_mul(m1[:], w[:], hh[:])
            hT_new = hpool.tile([128, 64], F16, tag="hT")
            nc.vector.tensor_add(hT_new[:], m1[:], zh[:])
            hs = hist[:, (t % FLUSH) * 64: (t % FLUSH + 1) * 64]
            nc.vector.tensor_add(hs, m1[:], zh[:])
            if t % FLUSH == FLUSH - 1 or t == T - 1:
                base = t - (t % FLUSH)
                nc.sync.dma_start(
                    out_d[:, base * 64: (t + 1) * 64],
                    hist[:, : (t % FLUSH + 1) * 64])

            m1_prev = m1
            zh_prev = zh
            hT_prev = hT_new
            hprev = hs

    nc.compile()
    return nc


def _u_layout(M):
    # [U, U] -> [128, KC*U]: out[p, k*U+u] = M[k*128+p, u]
    return np.ascontiguousarray(
        np.asarray(M, np.float32).reshape(KC, 128, U).transpose(1, 0, 2)
        .reshape(128, KC * U)).astype(np.float16)


def _w_layout(M):
    # [D, U] -> [128, DC*U]: out[p, dc*U+u] = M[dc*128+p, u]
    return np.ascontiguousarray(
        np.asarray(M, np.float32).reshape(DC, 128, U).transpose(1, 0, 2)
        .reshape(128, DC * U)).astype(np.float16)


def prepare(inputs, Wz, Uz, bz, Wr, Ur, br, Wh, Uh, bh, T):
    """Build the Bass program and the per-core input maps."""
    x = np.asarray(inputs, dtype=np.float32)[:, :T, :]

    uz, ur, uh = _u_layout(Uz), _u_layout(Ur), _u_layout(Uh)
    wz, wr, wh = _w_layout(Wz), _w_layout(Wr), _w_layout(Wh)
    b8 = np.concatenate(
        [np.asarray(v, np.float32).reshape(8, 128) for v in (bz, br, bh)],
        axis=1).astype(np.float16)
    e8 = np.kron(np.eye(8, dtype=np.float16), np.ones((1, 8), np.float16))
    e8 = np.ascontiguousarray(e8)

    nc = bacc.Bacc("TRN2", target_bir_lowering=False, debug=False,
                   num_devices=N_CORES)
    build(nc, T)

    in_maps = []
    for c in range(N_CORES):
        xc = x[c * BS:(c + 1) * BS]               # [BS, T, D]
        # xt[p, dc, t, b] = xc[b, t, dc*128+p]
        xtc = np.ascontiguousarray(
            xc.reshape(BS, T, DC, 128).transpose(3, 2, 1, 0)
            .reshape(128, DC * T * BS)).astype(np.float16)
        in_maps.append({
            "uz": uz, "ur": ur, "uh": uh, "wz": wz, "wr": wr, "wh": wh,
            "xt": xtc, "b8": b8, "e8": e8,
        })
    return nc, in_maps


def assemble(results):
    outs = []
    T = results[0]["out"].shape[1] // 64
    for c in range(N_CORES):
        o = results[c]["out"]                     # [128, T*64]
        # o[p, t*64 + uc*8 + b] = h_t[b, uc*128+p]
        o = o.reshape(128, T, NUC, BS).transpose(3, 1, 2, 0)  # [b, t, uc, p]
        outs.append(np.ascontiguousarray(o.reshape(BS, T, U)))
    return np.concatenate(outs, axis=0)           # [B, T, U]


def kernel(inputs, Wz, Uz, bz, Wr, Ur, br, Wh, Uh, bh, _T=None):
    T = inputs.shape[1] if _T is None else _T
    nc, in_maps = prepare(inputs, Wz, Uz, bz, Wr, Ur, br, Wh, Uh, bh, T)
    res = run_bass_kernel_spmd(nc, in_maps, list(range(N_CORES)))
    return assemble(res.results)
